# revision 34
# baseline (speedup 1.0000x reference)
"""Trainium2 Bass kernel for AuxiliaryMultiHeadedAttention.

Reference computation (B=4, S=1024, HID=1024, H=16 heads, DH=64):
    qh  = split_heads(q @ Wq.T + bq)
    kh  = split_heads(k @ Wk.T + bk)
    vh  = split_heads(v @ Wv.T + bv)
    kbh = split_heads(k_b @ Wkb.T + bkb)
    corr = qh @ (kh + kbh).T / sqrt(3*DH)
    corr = where(mask[b, t] == 0, -1e9, corr)          # mask over key positions
    prob = softmax(corr, axis=-1)
    out  = merge_heads(prob @ vh) @ Wo.T + bo

Sharding: 8 cores = 4 batches x 2 head-groups (8 heads each).  Each core
computes its batch's projections for its 8 heads, attention, and a partial
output projection over its 512 hidden dims.  Host sums the two partials per
batch (replaces the all-reduce) and adds bo.

Masked key positions contribute exactly zero (softmax of -1e9, V rows
zeroed), so the host compacts k/k_b/v to the mask's active positions, padded
to a multiple of 128 (TK).  All key-side stages (K/Kb/V projections, QK, PV)
scale with TK/S; for the graded mask (~52% active) TK=640, cutting PE work
~1.4x.  Padding tokens carry maskf=0 and are excluded by the same mask
multiply + fused-denominator path that handled masking before, so the
result is unchanged.

Device-side layout is feature-major ([feature, token]); the host feeds
pre-transposed activations and weights so no on-chip transposes are needed.
Scores are computed transposed ([t, s]); softmax over t is handled by
multiplying exp tiles against V extended with a mask column on the PE
(the 65th output row of the PV matmul is the softmax denominator), so no
partition-dim reductions are needed.  Matmul inputs are bf16 by default
(same PE rate as fp32r on TRN2 at moving-dim>=256, half the DMA bytes);
KERNEL_MM_DT=f32r|f32 selects alternatives.
"""

import math
import os

import numpy as np

import concourse.bass as bass
import concourse.mybir as mybir
import concourse.tile as tile
from concourse import bacc
from concourse.bass_utils import run_bass_kernel_spmd

B, S, HID, H = 4, 1024, 1024, 16
DH = HID // H            # 64
NCORES = 8
HPC = H // 2             # 8 heads per core
DPC = HPC * DH           # 512 hidden dims per core
P = 128
KT = HID // P            # 8 k-tiles (contraction over hid)
ST = S // P              # 8 s-tiles (queries)
NB = 512                 # matmul moving free dim (one PSUM bank of fp32)
SC = S // NB             # 2 s-chunks
DT = DPC // P            # 4 d'-tiles
F32 = mybir.dt.float32
SCALE = 1.0 / math.sqrt(3 * DH)

_MM_NAME = os.environ.get("KERNEL_MM_DT", "bf16")
_OUT_NAME = os.environ.get("KERNEL_OUT_DT", "bf16")
REPS_IN_NEFF = 1
STAGES = os.environ.get("KERNEL_STAGES", "ABC")
BUFS = {
    "acts": int(os.environ.get("KERNEL_BUFS_ACTS", "10")),
    "wts": int(os.environ.get("KERNEL_BUFS_WTS", "10")),
    "expp": int(os.environ.get("KERNEL_BUFS_EXPP", "6")),
    "ps_sc": int(os.environ.get("KERNEL_BUFS_PSSC", "2")),
    "ps_acc": int(os.environ.get("KERNEL_BUFS_PSACC", "4")),
}
MM_DT = {
    "f32r": mybir.dt.float32r,
    "bf16": mybir.dt.bfloat16,
    "f32": mybir.dt.float32,
}[_MM_NAME]
OUT_DT = {
    "bf16": mybir.dt.bfloat16,
    "f32": F32,
}[_OUT_NAME]
# Projection stages can run their matmuls in fp8e4m3 with DoubleRow perf
# mode ("1"=Q proj, "2"=K/Kb proj, "3"=V proj; attention and the output
# projection stay in MM_DT).  Disabled by default: measured on HW, fp8
# inputs put ~6-7% systematic error on the output (vs the 2e-2 budget)
# for only ~25% device-time gain (HW DoubleRow is 2x bf16, not the cost
# model's 4x).
FP8 = mybir.dt.float8e4
FP8_STAGES = set(os.environ.get("KERNEL_FP8_STAGES", ""))


def _proj_fp8(stage):
    return stage in FP8_STAGES


def _np_bf16():
    import ml_dtypes
    return ml_dtypes.bfloat16


def _to_mm(a):
    """float32 ndarray -> MM_DT ndarray (fast bf16 via int rounding)."""
    a = np.ascontiguousarray(a, dtype=np.float32)
    if _MM_NAME != "bf16":
        return a
    u = a.view(np.uint32)
    # round-to-nearest-even on the top 16 bits
    r = ((u + 0x7FFF + ((u >> 16) & 1)) >> 16).astype(np.uint16)
    return r.view(_np_bf16()).reshape(a.shape)


def build_module(reps=1, stk=ST):
    """Build the Bass module for TK = stk*128 compacted key positions."""
    global REPS_IN_NEFF
    REPS_IN_NEFF = reps
    TK = stk * P
    nc = bacc.Bacc(
        "TRN2",
        target_bir_lowering=False,
        debug=False,
        num_devices=NCORES,
    )
    io = {}

    def din(name, shape, dt=MM_DT):
        io[name] = nc.dram_tensor(name, shape, dt, kind="ExternalInput").ap()

    dt1 = FP8 if _proj_fp8("1") else MM_DT
    dt2 = FP8 if _proj_fp8("2") else MM_DT
    dt3 = FP8 if _proj_fp8("3") else MM_DT
    din("qT", [HID, S], dt1)
    din("kT", [HID, TK], dt2)
    din("kbT", [HID, TK], dt2)
    din("vT", [HID, TK], dt3)
    din("wqT", [HID, DPC], dt1)
    din("wkT", [HID, DPC], dt2)
    din("wkbT", [HID, DPC], dt2)
    din("wvT", [HID, DPC], dt3)
    din("woT", [DPC, HID])
    din("bq", [DPC], F32)
    din("bks", [DPC], F32)    # bk + bkb, summed on host
    din("maskf", [TK], F32)   # compacted mask as float (0 only on padding)
    io["out"] = nc.dram_tensor("out", [S, HID], OUT_DT, kind="ExternalOutput").ap()

    with tile.TileContext(nc) as tc:
        _build_kernel(tc, io, stk)
    nc.compile()
    return nc


def _build_kernel(tc, io, stk):
    from contextlib import ExitStack

    nc = tc.nc
    TK = stk * P
    # key-side moving chunks of <=NB (e.g. TK=640 -> [(0,512),(512,128)])
    kchunks = []
    off = 0
    while off < TK:
        w = min(NB, TK - off)
        kchunks.append((off, w))
        off += w

    with ExitStack() as ctx:
        ctx.enter_context(
            nc.allow_low_precision(reason="matmul inputs intentionally MM_DT")
        )
        singles = ctx.enter_context(tc.tile_pool(name="singles", bufs=1))
        wts = ctx.enter_context(tc.tile_pool(name="wts", bufs=BUFS["wts"]))
        acts = ctx.enter_context(tc.tile_pool(name="acts", bufs=BUFS["acts"]))
        expp = ctx.enter_context(tc.tile_pool(name="expp", bufs=BUFS["expp"]))
        outp = ctx.enter_context(tc.tile_pool(name="outp", bufs=3))
        smalls = ctx.enter_context(tc.tile_pool(name="smalls", bufs=int(os.environ.get("KERNEL_BUFS_SMALLS", "2"))))
        ps_sc = ctx.enter_context(tc.tile_pool(name="ps_sc", bufs=BUFS["ps_sc"], space="PSUM"))
        ps_acc = ctx.enter_context(tc.tile_pool(name="ps_acc", bufs=BUFS["ps_acc"], space="PSUM"))

        # Resident intermediates, feature-major.  All matmul inputs use MM_DT.
        # Split into per-block tiles so consumers depend only on the blocks
        # they read, not on every writer of one big tile.
        QHT = [singles.tile([P, S], MM_DT, tag=f"qht{r}", name=f"qht{r}")
               for r in range(DT)]                            # qh.T   [d', s]
        KSUMT = [singles.tile([P, TK], MM_DT, tag=f"ksumt{r}", name=f"ksumt{r}")
                 for r in range(DT)]                          # (kh+kbh).T
        # V + mask column, token-major: per t-tile, per head: 64 vh cols + mask
        VHM = [singles.tile([P, HPC, DH + 1], MM_DT, tag=f"vhm{t}", name=f"vhm{t}")
               for t in range(stk)]
        HT = [singles.tile([P, S], MM_DT, tag=f"ht{r}", name=f"ht{r}")
              for r in range(DT)]                             # hidden.T [d', s]

        # Constants (tiles only; DMAs are emitted inside the body after the
        # first weight loads so they don't occupy HWDGE at t=0)
        bq_s = singles.tile([P, DT], F32, tag="bq")
        bks_s = singles.tile([P, DT], F32, tag="bks")
        mask_c = singles.tile([P, stk], F32, tag="mask")

        env = dict(locals())
        for _rep in range(REPS_IN_NEFF):
            _build_body(tc, io, env, _rep == 0)


def _build_body(tc, io, env, first_rep=True):
    nc = tc.nc
    Exp = mybir.ActivationFunctionType.Exp
    singles = env["singles"]; wts = env["wts"]; acts = env["acts"]
    expp = env["expp"]; outp = env["outp"]; smalls = env["smalls"]
    ps_sc = env["ps_sc"]; ps_acc = env["ps_acc"]
    QHT = env["QHT"]; KSUMT = env["KSUMT"]; VHM = env["VHM"]; HT = env["HT"]
    bq_s = env["bq_s"]; bks_s = env["bks_s"]; mask_c = env["mask_c"]
    stk = env["stk"]; TK = env["TK"]; kchunks = env["kchunks"]
    if True:
        KH = KT // 2   # k-tiles per DMA half

        # DMA consolidation: the HWDGE descriptor-generation cost is ~625 ns
        # per DMA *instruction* regardless of size, and the queue is in-order,
        # so many small loads serialize into a supply bottleneck.  Each tensor
        # is loaded in two k-tile halves; accessors hand out per-k-tile views.
        # Matmul accumulation chains are split into lo/hi k-tile phases so
        # the PE starts on the lo halves while the hi halves stream in.

        def geom(stage):
            """(two, dtype, n_ktiles, n_lo, perf_mode) for a projection."""
            if _proj_fp8(stage):
                return (True, FP8, KT // 2, KT // 4,
                        mybir.MatmulPerfMode.DoubleRow)
            return (False, MM_DT, KT, KH, None)

        def make_loader(pool, name, off, width, tag, stage, splits=None):
            two, dt, ktn, kh, _ = geom(stage)
            if two:
                src = io[name].rearrange("(kt u p) s -> p kt u s", p=P, u=2)
            else:
                src = io[name].rearrange("(kt p) s -> p kt s", p=P)
            if splits is None:
                splits = (kh, ktn - kh)
            bases = [sum(splits[:i]) for i in range(len(splits))]
            ts = [None] * len(splits)

            def emit(h):
                n = splits[h]
                shape = [P, n, 2, width] if two else [P, n, width]
                t = pool.tile(shape, dt, tag=tag, name=f"{tag}_{name}{off}_{h}")
                if two:
                    nc.sync.dma_start(t, src[:, bases[h]:bases[h] + n, :,
                                             off:off + width])
                else:
                    nc.sync.dma_start(t, src[:, bases[h]:bases[h] + n,
                                             off:off + width])
                ts[h] = t

            def acc(kt, c0=None, c1=None):
                for h in range(len(splits)):
                    if bases[h] <= kt < bases[h] + splits[h]:
                        t, r = ts[h], kt - bases[h]
                        if c0 is None:
                            return t[:, r, :, :] if two else t[:, r, :]
                        return (t[:, r, :, c0:c1] if two
                                else t[:, r, c0:c1])
            return emit, acc

        # ---- Stage A2: KSUMT[d', t] = Wk_g @ k.T + Wkb_g @ k_b.T + bks ----
        w0 = kchunks[0][1]
        two2, _, ktn2, kh2, pm2 = geom("2")
        wk_e, wk = make_loader(wts, "wkT", 0, DPC, "w", "2")
        kc_e, kc = make_loader(acts, "kT", 0, w0, "act", "2")
        wkb_e, wkb = make_loader(wts, "wkbT", 0, DPC, "w", "2")
        kbc_e, kbc = make_loader(acts, "kbT", 0, w0, "act", "2")
        wk_e(0)
        kc_e(0)
        wkb_e(0)
        kbc_e(0)
        if first_rep:
            nc.sync.dma_start(bq_s, io["bq"].rearrange("(t p) -> p t", p=P))
            nc.sync.dma_start(bks_s, io["bks"].rearrange("(t p) -> p t", p=P))
            nc.sync.dma_start(mask_c,
                              io["maskf"].rearrange("(t p) -> p t", p=P))
        wk_e(1)
        kc_e(1)
        wkb_e(1)
        kbc_e(1)
        if len(kchunks) > 1:
            off1, w1 = kchunks[1]
            kc1_e, kc1 = make_loader(acts, "kT", off1, w1, "act", "2",
                                     splits=(ktn2,))
            kbc1_e, kbc1 = make_loader(acts, "kbT", off1, w1, "act", "2",
                                       splits=(ktn2,))
            kc1_e(0)
            kbc1_e(0)

        # c0: phase-split accumulation, dt-major within each supply phase
        pss = [ps_acc.tile([P, NB], F32, tag="ps1", name=f"psA2_{i}")
               for i in range(DT)]
        phases = [
            (wk, kc, 0, kh2, True, False),
            (wkb, kbc, 0, kh2, False, False),
            (wk, kc, kh2, ktn2, False, False),
            (wkb, kbc, kh2, ktn2, False, True),
        ]
        for w_, a_, k0, k1, is_first, is_last in phases:
            for dt_ in range(DT):
                for kt in range(k0, k1):
                    nc.tensor.matmul(
                        pss[dt_][:, 0:w0], lhsT=w_(kt, dt_ * P, (dt_ + 1) * P),
                        rhs=a_(kt), start=(is_first and kt == k0),
                        stop=(is_last and kt == k1 - 1), perf_mode=pm2)
                if is_last:
                    nc.vector.tensor_scalar_add(
                        KSUMT[dt_][:, 0:w0], pss[dt_][:, 0:w0],
                        bks_s[:, dt_:dt_ + 1])
        # c1 (small remainder chunk): simple chained accumulation
        if len(kchunks) > 1:
            for dt_ in range(DT):
                ps = ps_acc.tile([P, NB], F32, tag="ps1")
                for kt in range(ktn2):
                    nc.tensor.matmul(
                        ps[:, 0:w1], lhsT=wk(kt, dt_ * P, (dt_ + 1) * P),
                        rhs=kc1(kt), start=(kt == 0), stop=False,
                        perf_mode=pm2)
                for kt in range(ktn2):
                    nc.tensor.matmul(
                        ps[:, 0:w1], lhsT=wkb(kt, dt_ * P, (dt_ + 1) * P),
                        rhs=kbc1(kt), start=False, stop=(kt == ktn2 - 1),
                        perf_mode=pm2)
                nc.vector.tensor_scalar_add(
                    KSUMT[dt_][:, off1:off1 + w1], ps[:, 0:w1],
                    bks_s[:, dt_:dt_ + 1])

        # ---- Stage A1 (c=0): QHT[d', s] = (Wq_g @ q.T) + bq ----
        # Emission order sets DMA priority: Q chunk 0 (feeds the first QK/exp
        # wave), then all of V (the PV chain needs full VHM), then Q chunk 1.
        two1, _, ktn1, kh1, pm1 = geom("1")
        wq_e, wq = make_loader(wts, "wqT", 0, DPC, "w", "1")
        wq_e(0)
        wq_e(1)

        def a1_chunk(c):
            qc_e, qc = make_loader(acts, "qT", c * NB, NB, "act", "1")
            qc_e(0)
            qc_e(1)
            pss = []
            for dt_ in range(DT):
                ps = ps_acc.tile([P, NB], F32, tag="ps1")
                pss.append(ps)
                for kt in range(kh1):
                    nc.tensor.matmul(
                        ps, lhsT=wq(kt, dt_ * P, (dt_ + 1) * P), rhs=qc(kt),
                        start=(kt == 0), stop=False, perf_mode=pm1)
            for dt_ in range(DT):
                ps = pss[dt_]
                for kt in range(kh1, ktn1):
                    nc.tensor.matmul(
                        ps, lhsT=wq(kt, dt_ * P, (dt_ + 1) * P), rhs=qc(kt),
                        start=False, stop=(kt == ktn1 - 1), perf_mode=pm1)
                nc.vector.tensor_scalar_add(
                    QHT[dt_][:, c * NB:(c + 1) * NB], ps, bq_s[:, dt_:dt_ + 1])

        a1_chunk(0)

        # ---- Stage A3: VHM[t, h, 0:64] = (v.T_tile.T @ Wv.T + bv) * mask[t];
        #      VHM[t, h, 64] = mask[t] ----
        two3, _, ktn3, kh3, pm3 = geom("3")
        vc_e, vc = make_loader(acts, "vT", 0, TK, "act", "3")
        wv_e, wv = make_loader(wts, "wvT", 0, DPC, "w", "3")
        vc_e(0)
        wv_e(0)
        vc_e(1)
        wv_e(1)
        for base in range(0, stk, DT):
            group = list(range(base, min(base + DT, stk)))
            psg = {}
            for tt in group:
                ps = ps_acc.tile([P, NB], F32, tag="ps1")
                psg[tt] = ps
                # bv is separable: sum_t prob*(vh+bv) = PV/denom + bv, and
                # bv flows through the output projection as the constant row
                # bv @ Wo.T, which the host adds at gather time.
                for kt in range(kh3):
                    nc.tensor.matmul(
                        ps, lhsT=vc(kt, tt * P, (tt + 1) * P), rhs=wv(kt),
                        start=(kt == 0), stop=False, perf_mode=pm3)
            for tt in group:
                ps = psg[tt]
                for kt in range(kh3, ktn3):
                    nc.tensor.matmul(
                        ps, lhsT=vc(kt, tt * P, (tt + 1) * P), rhs=wv(kt),
                        start=False, stop=(kt == ktn3 - 1), perf_mode=pm3)
                nc.vector.tensor_scalar_mul(
                    VHM[tt][:, :, 0:DH],
                    ps.rearrange("p (h d) -> p h d", h=HPC),
                    mask_c[:, tt:tt + 1],
                )
                nc.vector.tensor_copy(
                    VHM[tt][:, :, DH:DH + 1],
                    mask_c[:, tt:tt + 1, None].to_broadcast((P, HPC, 1)),
                )

        a1_chunk(1)

        if "B" not in STAGES:
            return
        # ---- Stage B: attention; s-chunk outer (unblocks on half of QHT),
        #      head pairs inner (adjacent QK matmuls hit disjoint PE row
        #      groups: bases 0 and 64) ----
        # exp tiles split in halves along t so the first half releases to PV
        # while the second half's QK still runs.  H1 even so pss pairs don't
        # straddle the halves.
        H1 = min(stk, 2 * ((stk + 3) // 4))
        HSZ = [H1, stk - H1]
        wo_src = io["woT"].rearrange("(it p) j -> p it j", p=P)
        wo = []
        for c2 in range(SC):
            t = wts.tile([P, DT, NB], MM_DT, tag="w", name=f"w_wo_{c2}")
            nc.sync.dma_start(t, wo_src[:, :, c2 * NB:(c2 + 1) * NB])
            wo.append(t)
        def c_block(mt):
            split = (mt == ST - 1)
            ot = outp.tile([P, S], OUT_DT, tag="ot", name=f"ot{mt}")
            for c2 in range(SC):
                ps = ps_acc.tile([P, NB], F32, tag="ps1")
                for it in range(DT):
                    nc.tensor.matmul(
                        ps,
                        lhsT=HT[it][:, mt * P:(mt + 1) * P],
                        rhs=wo[c2][:, it, :],
                        start=(it == 0),
                        stop=(it == DT - 1),
                    )
                nc.vector.tensor_copy(ot[:, c2 * NB:(c2 + 1) * NB], ps)
                if split:
                    nc.sync.dma_start(
                        io["out"][mt * P:(mt + 1) * P,
                                  c2 * NB:(c2 + 1) * NB],
                        ot[:, c2 * NB:(c2 + 1) * NB])
            if not split:
                nc.sync.dma_start(io["out"][mt * P:(mt + 1) * P, :], ot)

        deferred = []
        for c in range(SC):
            for pr in range(HPC // 2):
                r = pr
                # In the last s-chunk, slot a deferred chunk-0 C block
                # between this pair's QK and PV: it is ready PE work that
                # fills the exp (ACT) latency of the final head-pairs.
                emit_deferred = (c == SC - 1 and deferred
                                 and pr >= HPC // 2 - len(deferred))
                exs = [
                    [expp.tile([P, HSZ[half], NB], MM_DT, tag="exp",
                               name=f"ex{c}_{pr}_{hh}_{half}")
                     for half in range(2) if HSZ[half]]
                    for hh in range(2)
                ]
                for jj in range((stk + 1) // 2):
                    js = [j for j in (2 * jj, 2 * jj + 1) if j < stk]
                    # High priority: the exp chain is the critical path; let
                    # QK matmuls preempt remaining projection matmuls so the
                    # ACT engine is fed as early as possible.
                    with tc.high_priority():
                        pss = [ps_sc.tile([P, 2, NB], F32, tag="ps2",
                                          name=f"ps2_{jj}_{i}") for i in range(2)]
                        for u, j in enumerate(js):
                            for hh in range(2):
                                bp = hh * DH
                                nc.tensor.matmul(
                                    pss[hh][:, u],
                                    lhsT=KSUMT[r][bp:bp + DH, j * P:(j + 1) * P],
                                    rhs=QHT[r][bp:bp + DH, c * NB:(c + 1) * NB],
                                    start=True,
                                    stop=True,
                                )
                        half = 0 if js[0] < H1 else 1
                        base = js[0] - half * H1
                        for hh in range(2):
                            nc.scalar.activation(
                                exs[hh][half][:, base:base + len(js), :],
                                pss[hh][:, 0:len(js)], Exp,
                                bias=0.0, scale=SCALE,
                            )
                if emit_deferred:
                    c_block(deferred.pop(0))
                for hh in range(2):
                    h = 2 * pr + hh
                    bp = hh * DH
                    # PV with fused denominator (65th row = sum_t exp * mask)
                    psh = ps_acc.tile([P, NB], F32, tag="ps1")
                    for j in range(stk):
                        half = 0 if j < H1 else 1
                        nc.tensor.matmul(
                            psh[0:DH + 1, :],
                            lhsT=VHM[j][:, h, :],
                            rhs=exs[hh][half][:, j - half * H1, :],
                            start=(j == 0),
                            stop=(j == stk - 1),
                        )
                    rec = smalls.tile([1, NB], F32, tag="rec")
                    nc.vector.reciprocal(rec, psh[DH:DH + 1, :])
                    recb = smalls.tile([DH, NB], F32, tag="recb")
                    nc.gpsimd.partition_broadcast(recb, rec)
                    nc.vector.tensor_mul(
                        HT[r][bp:bp + DH, c * NB:(c + 1) * NB],
                        psh[0:DH, :],
                        recb,
                    )

            # ---- Stage C (half): out rows for this s-chunk ----
            # The last two row-blocks of chunk 0 are deferred into the last
            # chunk's B stage (see emit_deferred above).
            if "C" in STAGES:
                blocks = list(range(c * (ST // SC), (c + 1) * (ST // SC)))
                if SC > 1 and c == 0:
                    blocks, deferred[:] = blocks[:-2], blocks[-2:]
                for mt in blocks:
                    c_block(mt)


def _to_fp8(a):
    a = np.ascontiguousarray(a, dtype=np.float32)
    return a.astype(np.dtype(mybir.dt.np(FP8)))


def make_in_maps(inputs):
    """Compact keys to active mask positions; returns (in_maps, stk)."""
    inp = {k: np.asarray(v) for k, v in inputs.items()}
    q, k, v, k_b = inp["q"], inp["k"], inp["v"], inp["k_b"]
    mask = np.asarray(inp["mask"])
    f32 = np.float32
    idxs = [np.nonzero(mask[b])[0] for b in range(B)]
    nmax = max(1, max(len(ix) for ix in idxs))
    stk = (nmax + P - 1) // P
    TK = stk * P
    cvt1 = _to_fp8 if _proj_fp8("1") else _to_mm
    cvt2 = _to_fp8 if _proj_fp8("2") else _to_mm
    cvt3 = _to_fp8 if _proj_fp8("3") else _to_mm

    def compact(x, b, cvt):
        ix = idxs[b]
        out = np.zeros((HID, TK), np.float32)
        out[:, :len(ix)] = x[b].T[:, ix]
        return cvt(out)

    wq = cvt1(inp["Wq"].T)        # [HID, HID] col-sharded below
    wk = cvt2(inp["Wk"].T)
    wkb = cvt2(inp["Wkb"].T)
    wv = cvt3(inp["Wv"].T)
    wo = _to_mm(inp["Wo"].T)
    in_maps = []
    for b in range(B):
        qT = cvt1(q[b].T)
        kT = compact(k, b, cvt2)
        kbT = compact(k_b, b, cvt2)
        vT = compact(v, b, cvt3)
        maskf = np.zeros(TK, f32)
        maskf[:len(idxs[b])] = 1.0
        for g in range(2):
            hs = slice(g * DPC, (g + 1) * DPC)
            in_maps.append({
                "qT": qT,
                "kT": kT,
                "kbT": kbT,
                "vT": vT,
                "wqT": np.ascontiguousarray(wq[:, hs]),
                "wkT": np.ascontiguousarray(wk[:, hs]),
                "wkbT": np.ascontiguousarray(wkb[:, hs]),
                "wvT": np.ascontiguousarray(wv[:, hs]),
                "woT": np.ascontiguousarray(wo[hs, :]),
                "bq": np.ascontiguousarray(inp["bq"][hs], dtype=f32),
                "bks": np.ascontiguousarray(
                    inp["bk"][hs] + inp["bkb"][hs], dtype=f32),
                "maskf": maskf,
            })
    return in_maps, stk


def gather(results, bo, bv_wo):
    out = np.empty((B, S, HID), np.float32)
    const = (np.asarray(bo, dtype=np.float32)
             + bv_wo[0] + bv_wo[1])
    for b in range(B):
        out[b] = (results[2 * b]["out"].astype(np.float32)
                  + results[2 * b + 1]["out"].astype(np.float32)
                  + const)
    return out


def bv_wo_terms(inputs):
    bv = np.asarray(inputs["bv"], dtype=np.float64)
    wo = np.asarray(inputs["Wo"], dtype=np.float64)
    return [
        (bv[g * DPC:(g + 1) * DPC] @ wo[:, g * DPC:(g + 1) * DPC].T)
        .astype(np.float32)
        for g in range(2)
    ]


_modules = {}
_executors = {}


def get_module(stk=ST, reps=1):
    key = (stk, reps)
    if key not in _modules:
        _modules[key] = build_module(reps=reps, stk=stk)
    return _modules[key]


class _Executor:
    """Builds the SPMD PJRT executable once; later calls only move data."""

    def __init__(self, nc):
        import jax
        from jax.sharding import Mesh, PartitionSpec, NamedSharding
        from jax.experimental.shard_map import shard_map
        from concourse import bass2jax

        bass2jax.install_neuronx_cc_hook()
        self.jax = jax
        self.nc = nc
        pid = nc.partition_id_tensor.name if nc.partition_id_tensor else None
        in_names, out_names, out_avals, zeros = [], [], [], []
        for alloc in nc.m.functions[0].allocations:
            if not isinstance(alloc, mybir.MemoryLocationSet):
                continue
            name = alloc.memorylocations[0].name
            if alloc.kind == "ExternalInput":
                if name != pid:
                    in_names.append(name)
            elif alloc.kind == "ExternalOutput":
                out_names.append(name)
                shape = tuple(alloc.tensor_shape)
                dtype = mybir.dt.np(alloc.dtype)
                out_avals.append(jax.core.ShapedArray(shape, dtype))
                zeros.append(np.zeros(shape, dtype))
        self.in_names, self.out_names = in_names, out_names
        all_in = in_names + out_names + ([pid] if pid else [])

        def _body(*args):
            operands = list(args)
            if pid:
                operands.append(bass2jax.partition_id_tensor())
            return tuple(bass2jax._bass_exec_p.bind(
                *operands,
                out_avals=tuple(out_avals),
                in_names=tuple(all_in),
                out_names=tuple(out_names),
                lowering_input_output_aliases=(),
                sim_require_finite=True,
                sim_require_nnan=True,
                nc=nc,
            ))

        devices = jax.devices()[:NCORES]
        mesh = Mesh(np.asarray(devices), ("core",))
        spec = PartitionSpec("core")
        self.sharding = NamedSharding(mesh, spec)
        n_args = len(in_names) + len(out_names)
        self.fn = jax.jit(
            shard_map(_body, mesh=mesh, in_specs=(spec,) * n_args,
                      out_specs=(spec,) * len(out_names), check_rep=False),
            keep_unused=True,
        )
        self.zero_dev = [
            jax.device_put(
                np.zeros((NCORES * z.shape[0], *z.shape[1:]), z.dtype),
                self.sharding,
            )
            for z in zeros
        ]
        self.out_shapes = [tuple(a.shape) for a in out_avals]

    def run(self, in_maps):
        jax = self.jax
        dev_in = [
            jax.device_put(
                np.concatenate(
                    [np.asarray(in_maps[c][n]) for c in range(NCORES)], axis=0
                ),
                self.sharding,
            )
            for n in self.in_names
        ]
        outs = self.fn(*dev_in, *self.zero_dev)
        jax.block_until_ready(outs)
        results = []
        for c in range(NCORES):
            res = {}
            for i, n in enumerate(self.out_names):
                sh = self.out_shapes[i]
                res[n] = np.asarray(outs[i]).reshape(NCORES, *sh)[c]
            results.append(res)
        return results


def get_executor(stk=ST, reps=1):
    key = (stk, reps)
    if key not in _executors:
        _executors[key] = _Executor(get_module(stk, reps))
    return _executors[key]


def kernel(**inputs):
    global _executors
    in_maps, stk = make_in_maps(inputs)
    last_err = None
    for attempt in range(3):
        try:
            if attempt < 2:
                res = get_executor(stk).run(in_maps)
            else:
                # fall back to the stock runner path
                res = run_bass_kernel_spmd(
                    get_module(stk), in_maps, core_ids=list(range(NCORES))
                ).results
            return gather(res, inputs["bo"], bv_wo_terms(inputs))
        except Exception as e:  # transient NRT/device errors: rebuild + retry
            last_err = e
            _executors = {}
            import time as _time
            _time.sleep(2.0 * (attempt + 1))
    raise last_err


# revision 46
# speedup vs baseline: 1.1166x; 1.1166x over previous
"""Trainium2 Bass kernel for AuxiliaryMultiHeadedAttention.

Reference computation (B=4, S=1024, HID=1024, H=16 heads, DH=64):
    qh  = split_heads(q @ Wq.T + bq)
    kh  = split_heads(k @ Wk.T + bk)
    vh  = split_heads(v @ Wv.T + bv)
    kbh = split_heads(k_b @ Wkb.T + bkb)
    corr = qh @ (kh + kbh).T / sqrt(3*DH)
    corr = where(mask[b, t] == 0, -1e9, corr)          # mask over key positions
    prob = softmax(corr, axis=-1)
    out  = merge_heads(prob @ vh) @ Wo.T + bo

Sharding: 8 cores = 4 batches x 2 head-groups (8 heads each).  Each core
computes its batch's projections for its 8 heads, attention, and a partial
output projection over its 512 hidden dims.  Host sums the two partials per
batch (replaces the all-reduce) and adds bo.

Masked key positions contribute exactly zero (softmax of -1e9, V rows
zeroed), so the host compacts k/k_b/v to the mask's active positions, padded
to a multiple of 128 (TK).  All key-side stages (K/Kb/V projections, QK, PV)
scale with TK/S; for the graded mask (~52% active) TK=640, cutting PE work
~1.4x.  Padding tokens carry maskf=0 and are excluded by the same mask
multiply + fused-denominator path that handled masking before, so the
result is unchanged.

Device-side layout is feature-major ([feature, token]); the host feeds
pre-transposed activations and weights so no on-chip transposes are needed.
Scores are computed transposed ([t, s]); softmax over t is handled by
multiplying exp tiles against V extended with a mask column on the PE
(the 65th output row of the PV matmul is the softmax denominator), so no
partition-dim reductions are needed.  Matmul inputs are bf16 by default
(same PE rate as fp32r on TRN2 at moving-dim>=256, half the DMA bytes);
KERNEL_MM_DT=f32r|f32 selects alternatives.
"""

import math
import os

import numpy as np

import concourse.bass as bass
import concourse.mybir as mybir
import concourse.tile as tile
from concourse import bacc
from concourse.bass_utils import run_bass_kernel_spmd

B, S, HID, H = 4, 1024, 1024, 16
DH = HID // H            # 64
NCORES = 8
HPC = H // 2             # 8 heads per core
DPC = HPC * DH           # 512 hidden dims per core
P = 128
KT = HID // P            # 8 k-tiles (contraction over hid)
ST = S // P              # 8 s-tiles (queries)
NB = 512                 # matmul moving free dim (one PSUM bank of fp32)
SC = S // NB             # 2 s-chunks
DT = DPC // P            # 4 d'-tiles
F32 = mybir.dt.float32
SCALE = 1.0 / math.sqrt(3 * DH)

_MM_NAME = os.environ.get("KERNEL_MM_DT", "bf16")
_OUT_NAME = os.environ.get("KERNEL_OUT_DT", "bf16")
REPS_IN_NEFF = 1
STAGES = os.environ.get("KERNEL_STAGES", "ABC")
BUFS = {
    "acts": int(os.environ.get("KERNEL_BUFS_ACTS", "10")),
    "wts": int(os.environ.get("KERNEL_BUFS_WTS", "10")),
    "expp": int(os.environ.get("KERNEL_BUFS_EXPP", "6")),
    "ps_sc": int(os.environ.get("KERNEL_BUFS_PSSC", "2")),
    "ps_acc": int(os.environ.get("KERNEL_BUFS_PSACC", "4")),
}
MM_DT = {
    "f32r": mybir.dt.float32r,
    "bf16": mybir.dt.bfloat16,
    "f32": mybir.dt.float32,
}[_MM_NAME]
OUT_DT = {
    "bf16": mybir.dt.bfloat16,
    "f32": F32,
}[_OUT_NAME]
# Projection stages can run their matmuls in fp8e4m3 with DoubleRow perf
# mode ("1"=Q proj, "2"=K/Kb proj, "3"=V proj; attention and the output
# projection stay in MM_DT).  Disabled by default: measured on HW, fp8
# inputs put ~6-7% systematic error on the output (vs the 2e-2 budget)
# for only ~25% device-time gain (HW DoubleRow is 2x bf16, not the cost
# model's 4x).
FP8 = mybir.dt.float8e4
FP8_STAGES = set(os.environ.get("KERNEL_FP8_STAGES", ""))


def _proj_fp8(stage):
    return stage in FP8_STAGES


def _np_bf16():
    import ml_dtypes
    return ml_dtypes.bfloat16


def _to_mm(a):
    """float32 ndarray -> MM_DT ndarray (fast bf16 via int rounding)."""
    a = np.ascontiguousarray(a, dtype=np.float32)
    if _MM_NAME != "bf16":
        return a
    u = a.view(np.uint32)
    # round-to-nearest-even on the top 16 bits
    r = ((u + 0x7FFF + ((u >> 16) & 1)) >> 16).astype(np.uint16)
    return r.view(_np_bf16()).reshape(a.shape)


def build_module(reps=1, tk=S):
    """Build the Bass module for tk compacted key positions (32-granular;
    the last 128-tile may be partial)."""
    global REPS_IN_NEFF
    REPS_IN_NEFF = reps
    TK = tk
    stk = (TK + P - 1) // P
    nc = bacc.Bacc(
        "TRN2",
        target_bir_lowering=False,
        debug=False,
        num_devices=NCORES,
    )
    io = {}

    def din(name, shape, dt=MM_DT):
        io[name] = nc.dram_tensor(name, shape, dt, kind="ExternalInput").ap()

    dt1 = FP8 if _proj_fp8("1") else MM_DT
    dt2 = FP8 if _proj_fp8("2") else MM_DT
    dt3 = FP8 if _proj_fp8("3") else MM_DT
    din("qT", [HID, S], dt1)
    din("kT", [HID, TK], dt2)
    din("kbT", [HID, TK], dt2)
    din("vT", [HID, TK], dt3)
    # maskf stays padded to whole tiles (tile-indexed constant)
    din("wqT", [HID, DPC], dt1)
    din("wkT", [HID, DPC], dt2)
    din("wkbT", [HID, DPC], dt2)
    din("wvT", [HID, DPC], dt3)
    din("woT", [DPC, HID])
    din("bq", [DPC], F32)
    din("bks", [DPC], F32)    # bk + bkb, summed on host
    din("maskf", [stk * P], F32)  # compacted mask, 0 only on padding
    io["out"] = nc.dram_tensor("out", [S, HID], OUT_DT, kind="ExternalOutput").ap()

    with tile.TileContext(nc) as tc:
        _build_kernel(tc, io, TK)
    nc.compile()
    return nc


def _build_kernel(tc, io, tk):
    from contextlib import ExitStack

    nc = tc.nc
    TK = tk
    stk = (TK + P - 1) // P
    # per-t-tile widths; the last tile may be partial (32-granular)
    twid = [min(P, TK - t * P) for t in range(stk)]
    # key-side moving chunks of <=NB (e.g. TK=544 -> [(0,512),(512,32)])
    kchunks = []
    off = 0
    while off < TK:
        w = min(NB, TK - off)
        kchunks.append((off, w))
        off += w

    with ExitStack() as ctx:
        ctx.enter_context(
            nc.allow_low_precision(reason="matmul inputs intentionally MM_DT")
        )
        singles = ctx.enter_context(tc.tile_pool(name="singles", bufs=1))
        wts = ctx.enter_context(tc.tile_pool(name="wts", bufs=BUFS["wts"]))
        acts = ctx.enter_context(tc.tile_pool(name="acts", bufs=BUFS["acts"]))
        expp = ctx.enter_context(tc.tile_pool(name="expp", bufs=BUFS["expp"]))
        outp = ctx.enter_context(tc.tile_pool(name="outp", bufs=3))
        smalls = ctx.enter_context(tc.tile_pool(name="smalls", bufs=int(os.environ.get("KERNEL_BUFS_SMALLS", "2"))))
        ps_sc = ctx.enter_context(tc.tile_pool(name="ps_sc", bufs=BUFS["ps_sc"], space="PSUM"))
        ps_acc = ctx.enter_context(tc.tile_pool(name="ps_acc", bufs=BUFS["ps_acc"], space="PSUM"))

        # Resident intermediates, feature-major.  All matmul inputs use MM_DT.
        # Split into per-block tiles so consumers depend only on the blocks
        # they read, not on every writer of one big tile.
        QHT = [singles.tile([P, S], MM_DT, tag=f"qht{r}", name=f"qht{r}")
               for r in range(DT)]                            # qh.T   [d', s]
        KSUMT = [singles.tile([P, TK], MM_DT, tag=f"ksumt{r}", name=f"ksumt{r}")
                 for r in range(DT)]                          # (kh+kbh).T
        # V + mask column, token-major: per t-tile, per head: 64 vh cols + mask
        VHM = [singles.tile([P, HPC, DH + 1], MM_DT, tag=f"vhm{t}", name=f"vhm{t}")
               for t in range(stk)]
        HT = [singles.tile([P, S], MM_DT, tag=f"ht{r}", name=f"ht{r}")
              for r in range(DT)]                             # hidden.T [d', s]

        # Constants (tiles only; DMAs are emitted inside the body after the
        # first weight loads so they don't occupy HWDGE at t=0)
        bq_s = singles.tile([P, DT], F32, tag="bq")
        bks_s = singles.tile([P, DT], F32, tag="bks")
        mask_c = singles.tile([P, stk], F32, tag="mask")

        env = dict(locals())
        for _rep in range(REPS_IN_NEFF):
            _build_body(tc, io, env, _rep == 0)


def _build_body(tc, io, env, first_rep=True):
    nc = tc.nc
    Exp = mybir.ActivationFunctionType.Exp
    singles = env["singles"]; wts = env["wts"]; acts = env["acts"]
    expp = env["expp"]; outp = env["outp"]; smalls = env["smalls"]
    ps_sc = env["ps_sc"]; ps_acc = env["ps_acc"]
    QHT = env["QHT"]; KSUMT = env["KSUMT"]; VHM = env["VHM"]; HT = env["HT"]
    bq_s = env["bq_s"]; bks_s = env["bks_s"]; mask_c = env["mask_c"]
    stk = env["stk"]; TK = env["TK"]; kchunks = env["kchunks"]
    twid = env["twid"]
    if True:
        KH = KT // 2   # k-tiles per DMA half

        # DMA consolidation: the HWDGE descriptor-generation cost is ~625 ns
        # per DMA *instruction* regardless of size, and the queue is in-order,
        # so many small loads serialize into a supply bottleneck.  Each tensor
        # is loaded in two k-tile halves; accessors hand out per-k-tile views.
        # Matmul accumulation chains are split into lo/hi k-tile phases so
        # the PE starts on the lo halves while the hi halves stream in.

        def geom(stage):
            """(two, dtype, n_ktiles, n_lo, perf_mode) for a projection."""
            if _proj_fp8(stage):
                return (True, FP8, KT // 2, KT // 4,
                        mybir.MatmulPerfMode.DoubleRow)
            return (False, MM_DT, KT, KH, None)

        def make_loader(pool, name, off, width, tag, stage, splits=None):
            two, dt, ktn, kh, _ = geom(stage)
            if two:
                src = io[name].rearrange("(kt u p) s -> p kt u s", p=P, u=2)
            else:
                src = io[name].rearrange("(kt p) s -> p kt s", p=P)
            if splits is None:
                splits = (kh, ktn - kh)
            bases = [sum(splits[:i]) for i in range(len(splits))]
            ts = [None] * len(splits)

            def emit(h):
                n = splits[h]
                shape = [P, n, 2, width] if two else [P, n, width]
                t = pool.tile(shape, dt, tag=tag, name=f"{tag}_{name}{off}_{h}")
                if two:
                    nc.sync.dma_start(t, src[:, bases[h]:bases[h] + n, :,
                                             off:off + width])
                else:
                    nc.sync.dma_start(t, src[:, bases[h]:bases[h] + n,
                                             off:off + width])
                ts[h] = t

            def acc(kt, c0=None, c1=None):
                for h in range(len(splits)):
                    if bases[h] <= kt < bases[h] + splits[h]:
                        t, r = ts[h], kt - bases[h]
                        if c0 is None:
                            return t[:, r, :, :] if two else t[:, r, :]
                        return (t[:, r, :, c0:c1] if two
                                else t[:, r, c0:c1])
            return emit, acc

        # ---- Stage A2: KSUMT[d', t] = Wk_g @ k.T + Wkb_g @ k_b.T + bks ----
        w0 = kchunks[0][1]
        two2, _, ktn2, kh2, pm2 = geom("2")
        wk_e, wk = make_loader(wts, "wkT", 0, DPC, "w", "2")
        kc_e, kc = make_loader(acts, "kT", 0, w0, "act", "2")
        wkb_e, wkb = make_loader(wts, "wkbT", 0, DPC, "w", "2")
        kbc_e, kbc = make_loader(acts, "kbT", 0, w0, "act", "2")
        wk_e(0)
        kc_e(0)
        wkb_e(0)
        kbc_e(0)
        if first_rep:
            nc.sync.dma_start(bq_s, io["bq"].rearrange("(t p) -> p t", p=P))
            nc.sync.dma_start(bks_s, io["bks"].rearrange("(t p) -> p t", p=P))
            nc.sync.dma_start(mask_c,
                              io["maskf"].rearrange("(t p) -> p t", p=P))
        wk_e(1)
        kc_e(1)
        wkb_e(1)
        kbc_e(1)
        if len(kchunks) > 1:
            off1, w1 = kchunks[1]
            kc1_e, kc1 = make_loader(acts, "kT", off1, w1, "act", "2",
                                     splits=(ktn2,))
            kbc1_e, kbc1 = make_loader(acts, "kbT", off1, w1, "act", "2",
                                       splits=(ktn2,))
            kc1_e(0)
            kbc1_e(0)

        # c0: phase-split accumulation, dt-major within each supply phase
        pss = [ps_acc.tile([P, NB], F32, tag="ps1", name=f"psA2_{i}")
               for i in range(DT)]
        phases = [
            (wk, kc, 0, kh2, True, False),
            (wkb, kbc, 0, kh2, False, False),
            (wk, kc, kh2, ktn2, False, False),
            (wkb, kbc, kh2, ktn2, False, True),
        ]
        for w_, a_, k0, k1, is_first, is_last in phases:
            for dt_ in range(DT):
                for kt in range(k0, k1):
                    nc.tensor.matmul(
                        pss[dt_][:, 0:w0], lhsT=w_(kt, dt_ * P, (dt_ + 1) * P),
                        rhs=a_(kt), start=(is_first and kt == k0),
                        stop=(is_last and kt == k1 - 1), perf_mode=pm2)
                if is_last:
                    nc.vector.tensor_scalar_add(
                        KSUMT[dt_][:, 0:w0], pss[dt_][:, 0:w0],
                        bks_s[:, dt_:dt_ + 1])
        # c1 (small remainder chunk): simple chained accumulation
        if len(kchunks) > 1:
            for dt_ in range(DT):
                ps = ps_acc.tile([P, NB], F32, tag="ps1")
                for kt in range(ktn2):
                    nc.tensor.matmul(
                        ps[:, 0:w1], lhsT=wk(kt, dt_ * P, (dt_ + 1) * P),
                        rhs=kc1(kt), start=(kt == 0), stop=False,
                        perf_mode=pm2)
                for kt in range(ktn2):
                    nc.tensor.matmul(
                        ps[:, 0:w1], lhsT=wkb(kt, dt_ * P, (dt_ + 1) * P),
                        rhs=kbc1(kt), start=False, stop=(kt == ktn2 - 1),
                        perf_mode=pm2)
                nc.vector.tensor_scalar_add(
                    KSUMT[dt_][:, off1:off1 + w1], ps[:, 0:w1],
                    bks_s[:, dt_:dt_ + 1])

        # ---- Stage A1 (c=0): QHT[d', s] = (Wq_g @ q.T) + bq ----
        # Emission order sets DMA priority: Q chunk 0 (feeds the first QK/exp
        # wave), then all of V (the PV chain needs full VHM), then Q chunk 1.
        two1, _, ktn1, kh1, pm1 = geom("1")
        wq_e, wq = make_loader(wts, "wqT", 0, DPC, "w", "1")
        wq_e(0)
        wq_e(1)

        def a1_chunk(c):
            qc_e, qc = make_loader(acts, "qT", c * NB, NB, "act", "1")
            qc_e(0)
            qc_e(1)
            pss = []
            for dt_ in range(DT):
                ps = ps_acc.tile([P, NB], F32, tag="ps1")
                pss.append(ps)
                for kt in range(kh1):
                    nc.tensor.matmul(
                        ps, lhsT=wq(kt, dt_ * P, (dt_ + 1) * P), rhs=qc(kt),
                        start=(kt == 0), stop=False, perf_mode=pm1)
            for dt_ in range(DT):
                ps = pss[dt_]
                for kt in range(kh1, ktn1):
                    nc.tensor.matmul(
                        ps, lhsT=wq(kt, dt_ * P, (dt_ + 1) * P), rhs=qc(kt),
                        start=False, stop=(kt == ktn1 - 1), perf_mode=pm1)
                nc.vector.tensor_scalar_add(
                    QHT[dt_][:, c * NB:(c + 1) * NB], ps, bq_s[:, dt_:dt_ + 1])

        a1_chunk(0)

        # ---- Stage A3: VHM[t, h, 0:64] = (v.T_tile.T @ Wv.T + bv) * mask[t];
        #      VHM[t, h, 64] = mask[t] ----
        two3, _, ktn3, kh3, pm3 = geom("3")
        vc_e, vc = make_loader(acts, "vT", 0, TK, "act", "3")
        wv_e, wv = make_loader(wts, "wvT", 0, DPC, "w", "3")
        vc_e(0)
        wv_e(0)
        vc_e(1)
        wv_e(1)
        for base in range(0, stk, DT):
            group = list(range(base, min(base + DT, stk)))
            psg = {}
            for tt in group:
                ps = ps_acc.tile([P, NB], F32, tag="ps1")
                psg[tt] = ps
                # bv is separable: sum_t prob*(vh+bv) = PV/denom + bv, and
                # bv flows through the output projection as the constant row
                # bv @ Wo.T, which the host adds at gather time.
                for kt in range(kh3):
                    nc.tensor.matmul(
                        ps[0:twid[tt], :],
                        lhsT=vc(kt, tt * P, tt * P + twid[tt]), rhs=wv(kt),
                        start=(kt == 0), stop=False, perf_mode=pm3)
            for tt in group:
                ps = psg[tt]
                w = twid[tt]
                for kt in range(kh3, ktn3):
                    nc.tensor.matmul(
                        ps[0:w, :], lhsT=vc(kt, tt * P, tt * P + w),
                        rhs=wv(kt),
                        start=False, stop=(kt == ktn3 - 1), perf_mode=pm3)
                nc.vector.tensor_scalar_mul(
                    VHM[tt][0:w, :, 0:DH],
                    ps[0:w, :].rearrange("p (h d) -> p h d", h=HPC),
                    mask_c[0:w, tt:tt + 1],
                )
                nc.vector.tensor_copy(
                    VHM[tt][0:w, :, DH:DH + 1],
                    mask_c[0:w, tt:tt + 1, None].to_broadcast((w, HPC, 1)),
                )

        a1_chunk(1)

        if "B" not in STAGES:
            return
        # ---- Stage B: attention; s-chunk outer (unblocks on half of QHT),
        #      head pairs inner (adjacent QK matmuls hit disjoint PE row
        #      groups: bases 0 and 64) ----
        # exp tiles split in halves along t so the first half releases to PV
        # while the second half's QK still runs.  H1 even so pss pairs don't
        # straddle the halves.
        H1 = min(stk, 2 * ((stk + 3) // 4))
        HSZ = [H1, stk - H1]
        wo_src = io["woT"].rearrange("(it p) j -> p it j", p=P)
        wo = []
        for c2 in range(SC):
            t = wts.tile([P, DT, NB], MM_DT, tag="w", name=f"w_wo_{c2}")
            nc.sync.dma_start(t, wo_src[:, :, c2 * NB:(c2 + 1) * NB])
            wo.append(t)
        def c_block(mt):
            split = (mt == ST - 1)
            ot = outp.tile([P, S], OUT_DT, tag="ot", name=f"ot{mt}")
            for c2 in range(SC):
                ps = ps_acc.tile([P, NB], F32, tag="ps1")
                for it in range(DT):
                    nc.tensor.matmul(
                        ps,
                        lhsT=HT[it][:, mt * P:(mt + 1) * P],
                        rhs=wo[c2][:, it, :],
                        start=(it == 0),
                        stop=(it == DT - 1),
                    )
                nc.vector.tensor_copy(ot[:, c2 * NB:(c2 + 1) * NB], ps)
                if split:
                    nc.sync.dma_start(
                        io["out"][mt * P:(mt + 1) * P,
                                  c2 * NB:(c2 + 1) * NB],
                        ot[:, c2 * NB:(c2 + 1) * NB])
            if not split:
                nc.sync.dma_start(io["out"][mt * P:(mt + 1) * P, :], ot)

        deferred = []
        for c in range(SC):
            for pr in range(HPC // 2):
                r = pr
                # In the last s-chunk, slot a deferred chunk-0 C block
                # between this pair's QK and PV: it is ready PE work that
                # fills the exp (ACT) latency of the final head-pairs.
                emit_deferred = (c == SC - 1 and deferred
                                 and pr >= HPC // 2 - len(deferred))
                exs = [
                    [expp.tile([P, HSZ[half], NB], MM_DT, tag="exp",
                               name=f"ex{c}_{pr}_{hh}_{half}")
                     for half in range(2) if HSZ[half]]
                    for hh in range(2)
                ]
                # waves: pairs of full tiles; a partial tail tile rides alone
                waves, wi = [], 0
                while wi < stk:
                    if wi + 1 < stk and twid[wi + 1] == P:
                        waves.append([wi, wi + 1])
                        wi += 2
                    else:
                        waves.append([wi])
                        wi += 1
                for jj, js in enumerate(waves):
                    wj = twid[js[-1]]
                    # High priority: the exp chain is the critical path; let
                    # QK matmuls preempt remaining projection matmuls so the
                    # ACT engine is fed as early as possible.
                    with tc.high_priority():
                        pss = [ps_sc.tile([P, 2, NB], F32, tag="ps2",
                                          name=f"ps2_{jj}_{i}") for i in range(2)]
                        for u, j in enumerate(js):
                            for hh in range(2):
                                bp = hh * DH
                                nc.tensor.matmul(
                                    pss[hh][0:twid[j], u],
                                    lhsT=KSUMT[r][bp:bp + DH,
                                                  j * P:j * P + twid[j]],
                                    rhs=QHT[r][bp:bp + DH, c * NB:(c + 1) * NB],
                                    start=True,
                                    stop=True,
                                )
                        half = 0 if js[0] < H1 else 1
                        base = js[0] - half * H1
                        for hh in range(2):
                            nc.scalar.activation(
                                exs[hh][half][0:wj, base:base + len(js), :],
                                pss[hh][0:wj, 0:len(js)], Exp,
                                bias=0.0, scale=SCALE,
                            )
                if emit_deferred:
                    c_block(deferred.pop(0))
                for hh in range(2):
                    h = 2 * pr + hh
                    bp = hh * DH
                    # PV with fused denominator (65th row = sum_t exp * mask)
                    psh = ps_acc.tile([P, NB], F32, tag="ps1")
                    for j in range(stk):
                        half = 0 if j < H1 else 1
                        nc.tensor.matmul(
                            psh[0:DH + 1, :],
                            lhsT=VHM[j][0:twid[j], h, :],
                            rhs=exs[hh][half][0:twid[j], j - half * H1, :],
                            start=(j == 0),
                            stop=(j == stk - 1),
                        )
                    rec = smalls.tile([1, NB], F32, tag="rec")
                    nc.vector.reciprocal(rec, psh[DH:DH + 1, :])
                    recb = smalls.tile([DH, NB], F32, tag="recb")
                    nc.gpsimd.partition_broadcast(recb, rec)
                    nc.vector.tensor_mul(
                        HT[r][bp:bp + DH, c * NB:(c + 1) * NB],
                        psh[0:DH, :],
                        recb,
                    )

            # ---- Stage C (half): out rows for this s-chunk ----
            # The last two row-blocks of chunk 0 are deferred into the last
            # chunk's B stage (see emit_deferred above).
            if "C" in STAGES:
                blocks = list(range(c * (ST // SC), (c + 1) * (ST // SC)))
                if SC > 1 and c == 0:
                    blocks, deferred[:] = blocks[:-2], blocks[-2:]
                for mt in blocks:
                    c_block(mt)


def _to_fp8(a):
    a = np.ascontiguousarray(a, dtype=np.float32)
    return a.astype(np.dtype(mybir.dt.np(FP8)))


def make_in_maps(inputs):
    """Compact keys to active mask positions; returns (in_maps, stk)."""
    inp = {k: np.asarray(v) for k, v in inputs.items()}
    q, k, v, k_b = inp["q"], inp["k"], inp["v"], inp["k_b"]
    mask = np.asarray(inp["mask"])
    f32 = np.float32
    idxs = [np.nonzero(mask[b])[0] for b in range(B)]
    nmax = max(1, max(len(ix) for ix in idxs))
    TK = max(P, ((nmax + 31) // 32) * 32)
    stk = (TK + P - 1) // P
    cvt1 = _to_fp8 if _proj_fp8("1") else _to_mm
    cvt2 = _to_fp8 if _proj_fp8("2") else _to_mm
    cvt3 = _to_fp8 if _proj_fp8("3") else _to_mm

    def compact(x, b, cvt):
        ix = idxs[b]
        out = np.zeros((HID, TK), np.float32)
        out[:, :len(ix)] = x[b].T[:, ix]
        return cvt(out)

    wq = cvt1(inp["Wq"].T)        # [HID, HID] col-sharded below
    wk = cvt2(inp["Wk"].T)
    wkb = cvt2(inp["Wkb"].T)
    wv = cvt3(inp["Wv"].T)
    wo = _to_mm(inp["Wo"].T)
    in_maps = []
    for b in range(B):
        qT = cvt1(q[b].T)
        kT = compact(k, b, cvt2)
        kbT = compact(k_b, b, cvt2)
        vT = compact(v, b, cvt3)
        maskf = np.zeros(stk * P, f32)
        maskf[:len(idxs[b])] = 1.0
        for g in range(2):
            hs = slice(g * DPC, (g + 1) * DPC)
            in_maps.append({
                "qT": qT,
                "kT": kT,
                "kbT": kbT,
                "vT": vT,
                "wqT": np.ascontiguousarray(wq[:, hs]),
                "wkT": np.ascontiguousarray(wk[:, hs]),
                "wkbT": np.ascontiguousarray(wkb[:, hs]),
                "wvT": np.ascontiguousarray(wv[:, hs]),
                "woT": np.ascontiguousarray(wo[hs, :]),
                "bq": np.ascontiguousarray(inp["bq"][hs], dtype=f32),
                "bks": np.ascontiguousarray(
                    inp["bk"][hs] + inp["bkb"][hs], dtype=f32),
                "maskf": maskf,
            })
    return in_maps, TK


def gather(results, bo, bv_wo):
    out = np.empty((B, S, HID), np.float32)
    const = (np.asarray(bo, dtype=np.float32)
             + bv_wo[0] + bv_wo[1])
    for b in range(B):
        out[b] = (results[2 * b]["out"].astype(np.float32)
                  + results[2 * b + 1]["out"].astype(np.float32)
                  + const)
    return out


def bv_wo_terms(inputs):
    bv = np.asarray(inputs["bv"], dtype=np.float64)
    wo = np.asarray(inputs["Wo"], dtype=np.float64)
    return [
        (bv[g * DPC:(g + 1) * DPC] @ wo[:, g * DPC:(g + 1) * DPC].T)
        .astype(np.float32)
        for g in range(2)
    ]


_modules = {}
_executors = {}


def get_module(tk=S, reps=1):
    key = (tk, reps)
    if key not in _modules:
        _modules[key] = build_module(reps=reps, tk=tk)
    return _modules[key]


class _Executor:
    """Builds the SPMD PJRT executable once; later calls only move data."""

    def __init__(self, nc):
        import jax
        from jax.sharding import Mesh, PartitionSpec, NamedSharding
        from jax.experimental.shard_map import shard_map
        from concourse import bass2jax

        bass2jax.install_neuronx_cc_hook()
        self.jax = jax
        self.nc = nc
        pid = nc.partition_id_tensor.name if nc.partition_id_tensor else None
        in_names, out_names, out_avals, zeros = [], [], [], []
        for alloc in nc.m.functions[0].allocations:
            if not isinstance(alloc, mybir.MemoryLocationSet):
                continue
            name = alloc.memorylocations[0].name
            if alloc.kind == "ExternalInput":
                if name != pid:
                    in_names.append(name)
            elif alloc.kind == "ExternalOutput":
                out_names.append(name)
                shape = tuple(alloc.tensor_shape)
                dtype = mybir.dt.np(alloc.dtype)
                out_avals.append(jax.core.ShapedArray(shape, dtype))
                zeros.append(np.zeros(shape, dtype))
        self.in_names, self.out_names = in_names, out_names
        all_in = in_names + out_names + ([pid] if pid else [])

        def _body(*args):
            operands = list(args)
            if pid:
                operands.append(bass2jax.partition_id_tensor())
            return tuple(bass2jax._bass_exec_p.bind(
                *operands,
                out_avals=tuple(out_avals),
                in_names=tuple(all_in),
                out_names=tuple(out_names),
                lowering_input_output_aliases=(),
                sim_require_finite=True,
                sim_require_nnan=True,
                nc=nc,
            ))

        devices = jax.devices()[:NCORES]
        mesh = Mesh(np.asarray(devices), ("core",))
        spec = PartitionSpec("core")
        self.sharding = NamedSharding(mesh, spec)
        n_args = len(in_names) + len(out_names)
        self.fn = jax.jit(
            shard_map(_body, mesh=mesh, in_specs=(spec,) * n_args,
                      out_specs=(spec,) * len(out_names), check_rep=False),
            keep_unused=True,
        )
        self.zero_dev = [
            jax.device_put(
                np.zeros((NCORES * z.shape[0], *z.shape[1:]), z.dtype),
                self.sharding,
            )
            for z in zeros
        ]
        self.out_shapes = [tuple(a.shape) for a in out_avals]

    def run(self, in_maps):
        jax = self.jax
        dev_in = [
            jax.device_put(
                np.concatenate(
                    [np.asarray(in_maps[c][n]) for c in range(NCORES)], axis=0
                ),
                self.sharding,
            )
            for n in self.in_names
        ]
        outs = self.fn(*dev_in, *self.zero_dev)
        jax.block_until_ready(outs)
        results = []
        for c in range(NCORES):
            res = {}
            for i, n in enumerate(self.out_names):
                sh = self.out_shapes[i]
                res[n] = np.asarray(outs[i]).reshape(NCORES, *sh)[c]
            results.append(res)
        return results


def get_executor(tk=S, reps=1):
    key = (tk, reps)
    if key not in _executors:
        _executors[key] = _Executor(get_module(tk, reps))
    return _executors[key]


def kernel(**inputs):
    global _executors
    in_maps, tk = make_in_maps(inputs)
    last_err = None
    for attempt in range(3):
        try:
            if attempt < 2:
                res = get_executor(tk).run(in_maps)
            else:
                # fall back to the stock runner path
                res = run_bass_kernel_spmd(
                    get_module(tk), in_maps, core_ids=list(range(NCORES))
                ).results
            return gather(res, inputs["bo"], bv_wo_terms(inputs))
        except Exception as e:  # transient NRT/device errors: rebuild + retry
            last_err = e
            _executors = {}
            import time as _time
            _time.sleep(2.0 * (attempt + 1))
    raise last_err


# revision 51
# speedup vs baseline: 1.3817x; 1.2375x over previous
"""Trainium2 Bass kernel for AuxiliaryMultiHeadedAttention.

Reference computation (B=4, S=1024, HID=1024, H=16 heads, DH=64):
    qh  = split_heads(q @ Wq.T + bq)
    kh  = split_heads(k @ Wk.T + bk)
    vh  = split_heads(v @ Wv.T + bv)
    kbh = split_heads(k_b @ Wkb.T + bkb)
    corr = qh @ (kh + kbh).T / sqrt(3*DH)
    corr = where(mask[b, t] == 0, -1e9, corr)          # mask over key positions
    prob = softmax(corr, axis=-1)
    out  = merge_heads(prob @ vh) @ Wo.T + bo

Sharding: 8 cores = 4 batches x 2 head-groups (8 heads each).  Each core
computes its batch's projections for its 8 heads, attention, and a partial
output projection over its 512 hidden dims.  Host sums the two partials per
batch (replaces the all-reduce) and adds bo.

Masked key positions contribute exactly zero (softmax of -1e9, V rows
zeroed), so the host compacts k/k_b/v to the mask's active positions, padded
to a multiple of 128 (TK).  All key-side stages (K/Kb/V projections, QK, PV)
scale with TK/S; for the graded mask (~52% active) TK=640, cutting PE work
~1.4x.  Padding tokens carry maskf=0 and are excluded by the same mask
multiply + fused-denominator path that handled masking before, so the
result is unchanged.

Device-side layout is feature-major ([feature, token]); the host feeds
pre-transposed activations and weights so no on-chip transposes are needed.
Scores are computed transposed ([t, s]); softmax over t is handled by
multiplying exp tiles against V extended with a mask column on the PE
(the 65th output row of the PV matmul is the softmax denominator), so no
partition-dim reductions are needed.  Matmul inputs are bf16 by default
(same PE rate as fp32r on TRN2 at moving-dim>=256, half the DMA bytes);
KERNEL_MM_DT=f32r|f32 selects alternatives.
"""

import math
import os

import numpy as np

import concourse.bass as bass
import concourse.mybir as mybir
import concourse.tile as tile
from concourse import bacc
from concourse.bass_utils import run_bass_kernel_spmd

B, S, HID, H = 4, 1024, 1024, 16
DH = HID // H            # 64
NCORES = 8
HPC = H // 2             # 8 heads per core
DPC = HPC * DH           # 512 hidden dims per core
P = 128
KT = HID // P            # 8 k-tiles (contraction over hid)
ST = S // P              # 8 s-tiles (queries)
NB = 512                 # matmul moving free dim (one PSUM bank of fp32)
SC = S // NB             # 2 s-chunks
DT = DPC // P            # 4 d'-tiles
F32 = mybir.dt.float32
SCALE = 1.0 / math.sqrt(3 * DH)

_MM_NAME = os.environ.get("KERNEL_MM_DT", "bf16")
_OUT_NAME = os.environ.get("KERNEL_OUT_DT", "bf16")
REPS_IN_NEFF = 1
STAGES = os.environ.get("KERNEL_STAGES", "ABC")
BUFS = {
    "acts": int(os.environ.get("KERNEL_BUFS_ACTS", "10")),
    "wts": int(os.environ.get("KERNEL_BUFS_WTS", "10")),
    "expp": int(os.environ.get("KERNEL_BUFS_EXPP", "6")),
    "ps_sc": int(os.environ.get("KERNEL_BUFS_PSSC", "2")),
    "ps_acc": int(os.environ.get("KERNEL_BUFS_PSACC", "4")),
}
MM_DT = {
    "f32r": mybir.dt.float32r,
    "bf16": mybir.dt.bfloat16,
    "f32": mybir.dt.float32,
}[_MM_NAME]
OUT_DT = {
    "bf16": mybir.dt.bfloat16,
    "f32": F32,
}[_OUT_NAME]
# Projection stages can run their matmuls in fp8e4m3 with DoubleRow perf
# mode ("1"=Q proj, "2"=K/Kb proj, "3"=V proj; attention and the output
# projection stay in MM_DT).  Disabled by default: measured on HW, fp8
# inputs put ~6-7% systematic error on the output (vs the 2e-2 budget)
# for only ~25% device-time gain (HW DoubleRow is 2x bf16, not the cost
# model's 4x).
FP8 = mybir.dt.float8e4
FP8_STAGES = set(os.environ.get("KERNEL_FP8_STAGES", ""))


def _proj_fp8(stage):
    return stage in FP8_STAGES


def _np_bf16():
    import ml_dtypes
    return ml_dtypes.bfloat16


def _to_mm(a):
    """float32 ndarray -> MM_DT ndarray (fast bf16 via int rounding)."""
    a = np.ascontiguousarray(a, dtype=np.float32)
    if _MM_NAME != "bf16":
        return a
    u = a.view(np.uint32)
    # round-to-nearest-even on the top 16 bits
    r = ((u + 0x7FFF + ((u >> 16) & 1)) >> 16).astype(np.uint16)
    return r.view(_np_bf16()).reshape(a.shape)


def build_module(reps=1, tk=S):
    """Build the Bass module for tk compacted key positions (32-granular;
    the last 128-tile may be partial)."""
    global REPS_IN_NEFF
    REPS_IN_NEFF = reps
    TK = tk
    stk = (TK + P - 1) // P
    nc = bacc.Bacc(
        "TRN2",
        target_bir_lowering=False,
        debug=False,
        num_devices=NCORES,
    )
    io = {}

    def din(name, shape, dt=MM_DT):
        io[name] = nc.dram_tensor(name, shape, dt, kind="ExternalInput").ap()

    dt1 = FP8 if _proj_fp8("1") else MM_DT
    dt2 = FP8 if _proj_fp8("2") else MM_DT
    dt3 = FP8 if _proj_fp8("3") else MM_DT
    din("qT", [HID, S], dt1)
    din("kT", [HID, TK], dt2)
    din("kbT", [HID, TK], dt2)
    din("vT", [HID, TK], dt3)
    # maskf stays padded to whole tiles (tile-indexed constant)
    din("wqT", [HID, DPC], dt1)
    din("wkT", [HID, DPC], dt2)
    din("wkbT", [HID, DPC], dt2)
    din("wvT", [HID, DPC], dt3)
    din("woT", [DPC, HID])
    din("bq", [DPC], F32)
    din("bks", [DPC], F32)    # bk + bkb, summed on host
    din("maskf", [stk * P], F32)  # compacted mask, 0 only on padding
    io["out"] = nc.dram_tensor("out", [S, HID], OUT_DT, kind="ExternalOutput").ap()

    with tile.TileContext(nc) as tc:
        _build_kernel(tc, io, TK)
    nc.compile()
    return nc


def _build_kernel(tc, io, tk):
    from contextlib import ExitStack

    nc = tc.nc
    TK = tk
    stk = (TK + P - 1) // P
    # per-t-tile widths; the last tile may be partial (32-granular)
    twid = [min(P, TK - t * P) for t in range(stk)]
    # key-side moving chunks of <=NB (e.g. TK=544 -> [(0,512),(512,32)])
    kchunks = []
    off = 0
    while off < TK:
        w = min(NB, TK - off)
        kchunks.append((off, w))
        off += w

    with ExitStack() as ctx:
        ctx.enter_context(
            nc.allow_low_precision(reason="matmul inputs intentionally MM_DT")
        )
        singles = ctx.enter_context(tc.tile_pool(name="singles", bufs=1))
        wts = ctx.enter_context(tc.tile_pool(name="wts", bufs=BUFS["wts"]))
        acts = ctx.enter_context(tc.tile_pool(name="acts", bufs=BUFS["acts"]))
        expp = ctx.enter_context(tc.tile_pool(name="expp", bufs=BUFS["expp"]))
        outp = ctx.enter_context(tc.tile_pool(name="outp", bufs=3))
        smalls = ctx.enter_context(tc.tile_pool(name="smalls", bufs=int(os.environ.get("KERNEL_BUFS_SMALLS", "2"))))
        ps_sc = ctx.enter_context(tc.tile_pool(name="ps_sc", bufs=BUFS["ps_sc"], space="PSUM"))
        ps_acc = ctx.enter_context(tc.tile_pool(name="ps_acc", bufs=BUFS["ps_acc"], space="PSUM"))

        # Resident intermediates, feature-major.  All matmul inputs use MM_DT.
        # Split into per-block tiles so consumers depend only on the blocks
        # they read, not on every writer of one big tile.
        QHT = [singles.tile([P, S], MM_DT, tag=f"qht{r}", name=f"qht{r}")
               for r in range(DT)]                            # qh.T   [d', s]
        KSUMT = [singles.tile([P, TK], MM_DT, tag=f"ksumt{r}", name=f"ksumt{r}")
                 for r in range(DT)]                          # (kh+kbh).T
        # V + mask column, token-major: per t-tile, per head: 64 vh cols + mask
        VHM = [singles.tile([P, HPC, DH + 1], MM_DT, tag=f"vhm{t}", name=f"vhm{t}")
               for t in range(stk)]
        HT = [singles.tile([P, S], MM_DT, tag=f"ht{r}", name=f"ht{r}")
              for r in range(DT)]                             # hidden.T [d', s]

        # Constants (tiles only; DMAs are emitted inside the body after the
        # first weight loads so they don't occupy HWDGE at t=0)
        bq_s = singles.tile([P, DT], F32, tag="bq")
        bks_s = singles.tile([P, DT], F32, tag="bks")
        mask_c = singles.tile([P, stk], F32, tag="mask")

        env = dict(locals())
        for _rep in range(REPS_IN_NEFF):
            _build_body(tc, io, env, _rep == 0)


def _build_body(tc, io, env, first_rep=True):
    nc = tc.nc
    Exp = mybir.ActivationFunctionType.Exp
    singles = env["singles"]; wts = env["wts"]; acts = env["acts"]
    expp = env["expp"]; outp = env["outp"]; smalls = env["smalls"]
    ps_sc = env["ps_sc"]; ps_acc = env["ps_acc"]
    QHT = env["QHT"]; KSUMT = env["KSUMT"]; VHM = env["VHM"]; HT = env["HT"]
    bq_s = env["bq_s"]; bks_s = env["bks_s"]; mask_c = env["mask_c"]
    stk = env["stk"]; TK = env["TK"]; kchunks = env["kchunks"]
    twid = env["twid"]
    if True:
        KH = KT // 2   # k-tiles per DMA half

        # DMA consolidation: the HWDGE descriptor-generation cost is ~625 ns
        # per DMA *instruction* regardless of size, and the queue is in-order,
        # so many small loads serialize into a supply bottleneck.  Each tensor
        # is loaded in two k-tile halves; accessors hand out per-k-tile views.
        # Matmul accumulation chains are split into lo/hi k-tile phases so
        # the PE starts on the lo halves while the hi halves stream in.

        def geom(stage):
            """(two, dtype, n_ktiles, n_lo, perf_mode) for a projection."""
            if _proj_fp8(stage):
                return (True, FP8, KT // 2, KT // 4,
                        mybir.MatmulPerfMode.DoubleRow)
            return (False, MM_DT, KT, KH, None)

        def make_loader(pool, name, off, width, tag, stage, splits=None):
            two, dt, ktn, kh, _ = geom(stage)
            if two:
                src = io[name].rearrange("(kt u p) s -> p kt u s", p=P, u=2)
            else:
                src = io[name].rearrange("(kt p) s -> p kt s", p=P)
            if splits is None:
                splits = (kh, ktn - kh)
            bases = [sum(splits[:i]) for i in range(len(splits))]
            ts = [None] * len(splits)

            def emit(h):
                n = splits[h]
                shape = [P, n, 2, width] if two else [P, n, width]
                t = pool.tile(shape, dt, tag=tag, name=f"{tag}_{name}{off}_{h}")
                if two:
                    nc.sync.dma_start(t, src[:, bases[h]:bases[h] + n, :,
                                             off:off + width])
                else:
                    nc.sync.dma_start(t, src[:, bases[h]:bases[h] + n,
                                             off:off + width])
                ts[h] = t

            def acc(kt, c0=None, c1=None):
                for h in range(len(splits)):
                    if bases[h] <= kt < bases[h] + splits[h]:
                        t, r = ts[h], kt - bases[h]
                        if c0 is None:
                            return t[:, r, :, :] if two else t[:, r, :]
                        return (t[:, r, :, c0:c1] if two
                                else t[:, r, c0:c1])
            return emit, acc

        # ---- Stage A2: KSUMT[d', t] = Wk_g @ k.T + Wkb_g @ k_b.T + bks ----
        w0 = kchunks[0][1]
        two2, _, ktn2, kh2, pm2 = geom("2")
        wk_e, wk = make_loader(wts, "wkT", 0, DPC, "w", "2")
        kc_e, kc = make_loader(acts, "kT", 0, w0, "act", "2")
        wkb_e, wkb = make_loader(wts, "wkbT", 0, DPC, "w", "2")
        kbc_e, kbc = make_loader(acts, "kbT", 0, w0, "act", "2")
        wk_e(0)
        kc_e(0)
        wkb_e(0)
        kbc_e(0)
        if first_rep:
            nc.sync.dma_start(bq_s, io["bq"].rearrange("(t p) -> p t", p=P))
            nc.sync.dma_start(bks_s, io["bks"].rearrange("(t p) -> p t", p=P))
            nc.sync.dma_start(mask_c,
                              io["maskf"].rearrange("(t p) -> p t", p=P))
        wk_e(1)
        kc_e(1)
        wkb_e(1)
        kbc_e(1)
        if len(kchunks) > 1:
            off1, w1 = kchunks[1]
            kc1_e, kc1 = make_loader(acts, "kT", off1, w1, "act", "2",
                                     splits=(ktn2,))
            kbc1_e, kbc1 = make_loader(acts, "kbT", off1, w1, "act", "2",
                                       splits=(ktn2,))
            kc1_e(0)
            kbc1_e(0)

        # c0: phase-split accumulation, dt-major within each supply phase
        pss = [ps_acc.tile([P, NB], F32, tag="ps1", name=f"psA2_{i}")
               for i in range(DT)]
        phases = [
            (wk, kc, 0, kh2, True, False),
            (wkb, kbc, 0, kh2, False, False),
            (wk, kc, kh2, ktn2, False, False),
            (wkb, kbc, kh2, ktn2, False, True),
        ]
        for w_, a_, k0, k1, is_first, is_last in phases:
            for dt_ in range(DT):
                for kt in range(k0, k1):
                    nc.tensor.matmul(
                        pss[dt_][:, 0:w0], lhsT=w_(kt, dt_ * P, (dt_ + 1) * P),
                        rhs=a_(kt), start=(is_first and kt == k0),
                        stop=(is_last and kt == k1 - 1), perf_mode=pm2)
                if is_last:
                    nc.vector.tensor_scalar_add(
                        KSUMT[dt_][:, 0:w0], pss[dt_][:, 0:w0],
                        bks_s[:, dt_:dt_ + 1])
        # c1 (small remainder chunk): simple chained accumulation
        if len(kchunks) > 1:
            for dt_ in range(DT):
                ps = ps_acc.tile([P, NB], F32, tag="ps1")
                for kt in range(ktn2):
                    nc.tensor.matmul(
                        ps[:, 0:w1], lhsT=wk(kt, dt_ * P, (dt_ + 1) * P),
                        rhs=kc1(kt), start=(kt == 0), stop=False,
                        perf_mode=pm2)
                for kt in range(ktn2):
                    nc.tensor.matmul(
                        ps[:, 0:w1], lhsT=wkb(kt, dt_ * P, (dt_ + 1) * P),
                        rhs=kbc1(kt), start=False, stop=(kt == ktn2 - 1),
                        perf_mode=pm2)
                nc.vector.tensor_scalar_add(
                    KSUMT[dt_][:, off1:off1 + w1], ps[:, 0:w1],
                    bks_s[:, dt_:dt_ + 1])

        # ---- Stage A1 (c=0): QHT[d', s] = (Wq_g @ q.T) + bq ----
        # Emission order sets DMA priority: Q chunk 0 (feeds the first QK/exp
        # wave), then all of V (the PV chain needs full VHM), then Q chunk 1.
        # Tail-tile QK head-packing: the partial last t-tile (TW<=64) still
        # costs a full 512-row moving pass per head (QK is moving-bound), so
        # build a block-diagonal stationary per head pair — head0's tail keys
        # in columns 0:TW against d-rows 0:64, head1's in TW:2TW against
        # 64:128 — and compute both heads' tail scores in ONE pass over the
        # pair's full 128 d'-rows.
        TW = twid[-1]
        pack_tail = stk >= 2 and TW <= DH
        t_off = (stk - 1) * P
        if pack_tail:
            bd = []
            for r_ in range(DT):
                t = singles.tile([P, 2 * TW], MM_DT, tag=f"bd{r_}",
                                 name=f"bd{r_}")
                nc.gpsimd.memset(t, 0.0)
                nc.vector.tensor_copy(t[0:DH, 0:TW],
                                      KSUMT[r_][0:DH, t_off:t_off + TW])
                nc.vector.tensor_copy(t[DH:P, TW:2 * TW],
                                      KSUMT[r_][DH:P, t_off:t_off + TW])
                bd.append(t)

        two1, _, ktn1, kh1, pm1 = geom("1")
        wq_e, wq = make_loader(wts, "wqT", 0, DPC, "w", "1")
        wq_e(0)
        wq_e(1)

        def a1_chunk(c):
            qc_e, qc = make_loader(acts, "qT", c * NB, NB, "act", "1")
            qc_e(0)
            qc_e(1)
            pss = []
            for dt_ in range(DT):
                ps = ps_acc.tile([P, NB], F32, tag="ps1")
                pss.append(ps)
                for kt in range(kh1):
                    nc.tensor.matmul(
                        ps, lhsT=wq(kt, dt_ * P, (dt_ + 1) * P), rhs=qc(kt),
                        start=(kt == 0), stop=False, perf_mode=pm1)
            for dt_ in range(DT):
                ps = pss[dt_]
                for kt in range(kh1, ktn1):
                    nc.tensor.matmul(
                        ps, lhsT=wq(kt, dt_ * P, (dt_ + 1) * P), rhs=qc(kt),
                        start=False, stop=(kt == ktn1 - 1), perf_mode=pm1)
                nc.vector.tensor_scalar_add(
                    QHT[dt_][:, c * NB:(c + 1) * NB], ps, bq_s[:, dt_:dt_ + 1])

        a1_chunk(0)

        # ---- Stage A3: VHM[t, h, 0:64] = (v.T_tile.T @ Wv.T + bv) * mask[t];
        #      VHM[t, h, 64] = mask[t] ----
        two3, _, ktn3, kh3, pm3 = geom("3")
        vc_e, vc = make_loader(acts, "vT", 0, TK, "act", "3")
        wv_e, wv = make_loader(wts, "wvT", 0, DPC, "w", "3")
        vc_e(0)
        wv_e(0)
        vc_e(1)
        wv_e(1)
        for base in range(0, stk, DT):
            group = list(range(base, min(base + DT, stk)))
            psg = {}
            for tt in group:
                ps = ps_acc.tile([P, NB], F32, tag="ps1")
                psg[tt] = ps
                # bv is separable: sum_t prob*(vh+bv) = PV/denom + bv, and
                # bv flows through the output projection as the constant row
                # bv @ Wo.T, which the host adds at gather time.
                for kt in range(kh3):
                    nc.tensor.matmul(
                        ps[0:twid[tt], :],
                        lhsT=vc(kt, tt * P, tt * P + twid[tt]), rhs=wv(kt),
                        start=(kt == 0), stop=False, perf_mode=pm3)
            for tt in group:
                ps = psg[tt]
                w = twid[tt]
                for kt in range(kh3, ktn3):
                    nc.tensor.matmul(
                        ps[0:w, :], lhsT=vc(kt, tt * P, tt * P + w),
                        rhs=wv(kt),
                        start=False, stop=(kt == ktn3 - 1), perf_mode=pm3)
                nc.vector.tensor_scalar_mul(
                    VHM[tt][0:w, :, 0:DH],
                    ps[0:w, :].rearrange("p (h d) -> p h d", h=HPC),
                    mask_c[0:w, tt:tt + 1],
                )
                nc.vector.tensor_copy(
                    VHM[tt][0:w, :, DH:DH + 1],
                    mask_c[0:w, tt:tt + 1, None].to_broadcast((w, HPC, 1)),
                )

        if pack_tail:
            # Odd heads' tail V block duplicated at partitions TW:2TW so the
            # PV matmul's base partitions match the packed exp rows.
            vhm4b = singles.tile([P, HPC // 2, DH + 1], MM_DT, tag="vhm4b",
                                 name="vhm4b")
            nc.sync.dma_start(vhm4b[TW:2 * TW, :, :],
                              VHM[stk - 1][0:TW, 1::2, :])

        a1_chunk(1)

        if "B" not in STAGES:
            return
        # ---- Stage B: attention; s-chunk outer (unblocks on half of QHT),
        #      head pairs inner (adjacent QK matmuls hit disjoint PE row
        #      groups: bases 0 and 64) ----
        # exp tiles split in halves along t so the first half releases to PV
        # while the second half's QK still runs.  H1 even so pss pairs don't
        # straddle the halves.
        H1 = min(stk, 2 * ((stk + 3) // 4))
        HSZ = [H1, stk - H1]
        wo_src = io["woT"].rearrange("(it p) j -> p it j", p=P)
        wo = []
        for c2 in range(SC):
            t = wts.tile([P, DT, NB], MM_DT, tag="w", name=f"w_wo_{c2}")
            nc.sync.dma_start(t, wo_src[:, :, c2 * NB:(c2 + 1) * NB])
            wo.append(t)
        def c_block(mt):
            split = (mt == ST - 1)
            ot = outp.tile([P, S], OUT_DT, tag="ot", name=f"ot{mt}")
            for c2 in range(SC):
                ps = ps_acc.tile([P, NB], F32, tag="ps1")
                for it in range(DT):
                    nc.tensor.matmul(
                        ps,
                        lhsT=HT[it][:, mt * P:(mt + 1) * P],
                        rhs=wo[c2][:, it, :],
                        start=(it == 0),
                        stop=(it == DT - 1),
                    )
                nc.vector.tensor_copy(ot[:, c2 * NB:(c2 + 1) * NB], ps)
                if split:
                    nc.sync.dma_start(
                        io["out"][mt * P:(mt + 1) * P,
                                  c2 * NB:(c2 + 1) * NB],
                        ot[:, c2 * NB:(c2 + 1) * NB])
            if not split:
                nc.sync.dma_start(io["out"][mt * P:(mt + 1) * P, :], ot)

        deferred = []
        for c in range(SC):
            for pr in range(HPC // 2):
                r = pr
                # In the last s-chunk, slot a deferred chunk-0 C block
                # between this pair's QK and PV: it is ready PE work that
                # fills the exp (ACT) latency of the final head-pairs.
                emit_deferred = (c == SC - 1 and deferred
                                 and pr >= HPC // 2 - len(deferred))
                exs = [
                    [expp.tile([P, HSZ[half], NB], MM_DT, tag="exp",
                               name=f"ex{c}_{pr}_{hh}_{half}")
                     for half in range(2) if HSZ[half]]
                    for hh in range(2)
                ]
                # waves: pairs of full tiles; a partial tail tile rides alone
                waves, wi = [], 0
                while wi < stk:
                    if wi + 1 < stk and twid[wi + 1] == P:
                        waves.append([wi, wi + 1])
                        wi += 2
                    else:
                        waves.append([wi])
                        wi += 1
                for jj, js in enumerate(waves):
                    wj = twid[js[-1]]
                    tail = pack_tail and js[-1] == stk - 1
                    half = 0 if js[0] < H1 else 1
                    base = js[0] - half * H1
                    # High priority: the exp chain is the critical path; let
                    # QK matmuls preempt remaining projection matmuls so the
                    # ACT engine is fed as early as possible.
                    with tc.high_priority():
                        if tail:
                            # one pass, both heads (block-diag stationary)
                            pst = ps_sc.tile([P, 2, NB], F32, tag="ps2",
                                             name=f"ps2t_{jj}")
                            nc.tensor.matmul(
                                pst[0:2 * TW, 0],
                                lhsT=bd[r],
                                rhs=QHT[r][:, c * NB:(c + 1) * NB],
                                start=True,
                                stop=True,
                            )
                            nc.scalar.activation(
                                exs[0][half][0:TW, base:base + 1, :],
                                pst[0:TW, 0:1], Exp, bias=0.0, scale=SCALE)
                            nc.scalar.activation(
                                exs[1][half][TW:2 * TW, base:base + 1, :],
                                pst[TW:2 * TW, 0:1], Exp,
                                bias=0.0, scale=SCALE)
                            continue
                        pss = [ps_sc.tile([P, 2, NB], F32, tag="ps2",
                                          name=f"ps2_{jj}_{i}") for i in range(2)]
                        for u, j in enumerate(js):
                            for hh in range(2):
                                bp = hh * DH
                                nc.tensor.matmul(
                                    pss[hh][0:twid[j], u],
                                    lhsT=KSUMT[r][bp:bp + DH,
                                                  j * P:j * P + twid[j]],
                                    rhs=QHT[r][bp:bp + DH, c * NB:(c + 1) * NB],
                                    start=True,
                                    stop=True,
                                )
                        for hh in range(2):
                            nc.scalar.activation(
                                exs[hh][half][0:wj, base:base + len(js), :],
                                pss[hh][0:wj, 0:len(js)], Exp,
                                bias=0.0, scale=SCALE,
                            )
                if emit_deferred:
                    c_block(deferred.pop(0))
                for hh in range(2):
                    h = 2 * pr + hh
                    bp = hh * DH
                    # PV with fused denominator (65th row = sum_t exp * mask)
                    psh = ps_acc.tile([P, NB], F32, tag="ps1")
                    for j in range(stk):
                        half = 0 if j < H1 else 1
                        if pack_tail and j == stk - 1 and hh == 1:
                            # odd head's tail exp/V live at partitions TW:2TW
                            lhs = vhm4b[TW:2 * TW, pr, :]
                            rhs_ = exs[1][half][TW:2 * TW, j - half * H1, :]
                        else:
                            lhs = VHM[j][0:twid[j], h, :]
                            rhs_ = exs[hh][half][0:twid[j], j - half * H1, :]
                        nc.tensor.matmul(
                            psh[0:DH + 1, :],
                            lhsT=lhs,
                            rhs=rhs_,
                            start=(j == 0),
                            stop=(j == stk - 1),
                        )
                    rec = smalls.tile([1, NB], F32, tag="rec")
                    nc.vector.reciprocal(rec, psh[DH:DH + 1, :])
                    recb = smalls.tile([DH, NB], F32, tag="recb")
                    nc.gpsimd.partition_broadcast(recb, rec)
                    nc.vector.tensor_mul(
                        HT[r][bp:bp + DH, c * NB:(c + 1) * NB],
                        psh[0:DH, :],
                        recb,
                    )

            # ---- Stage C (half): out rows for this s-chunk ----
            # The last two row-blocks of chunk 0 are deferred into the last
            # chunk's B stage (see emit_deferred above).
            if "C" in STAGES:
                blocks = list(range(c * (ST // SC), (c + 1) * (ST // SC)))
                if SC > 1 and c == 0:
                    blocks, deferred[:] = blocks[:-2], blocks[-2:]
                for mt in blocks:
                    c_block(mt)


def _to_fp8(a):
    a = np.ascontiguousarray(a, dtype=np.float32)
    return a.astype(np.dtype(mybir.dt.np(FP8)))


def make_in_maps(inputs):
    """Compact keys to active mask positions; returns (in_maps, stk)."""
    inp = {k: np.asarray(v) for k, v in inputs.items()}
    q, k, v, k_b = inp["q"], inp["k"], inp["v"], inp["k_b"]
    mask = np.asarray(inp["mask"])
    f32 = np.float32
    idxs = [np.nonzero(mask[b])[0] for b in range(B)]
    nmax = max(1, max(len(ix) for ix in idxs))
    TK = max(P, ((nmax + 31) // 32) * 32)
    stk = (TK + P - 1) // P
    cvt1 = _to_fp8 if _proj_fp8("1") else _to_mm
    cvt2 = _to_fp8 if _proj_fp8("2") else _to_mm
    cvt3 = _to_fp8 if _proj_fp8("3") else _to_mm

    def compact(x, b, cvt):
        ix = idxs[b]
        out = np.zeros((HID, TK), np.float32)
        out[:, :len(ix)] = x[b].T[:, ix]
        return cvt(out)

    wq = cvt1(inp["Wq"].T)        # [HID, HID] col-sharded below
    wk = cvt2(inp["Wk"].T)
    wkb = cvt2(inp["Wkb"].T)
    wv = cvt3(inp["Wv"].T)
    wo = _to_mm(inp["Wo"].T)
    in_maps = []
    for b in range(B):
        qT = cvt1(q[b].T)
        kT = compact(k, b, cvt2)
        kbT = compact(k_b, b, cvt2)
        vT = compact(v, b, cvt3)
        maskf = np.zeros(stk * P, f32)
        maskf[:len(idxs[b])] = 1.0
        for g in range(2):
            hs = slice(g * DPC, (g + 1) * DPC)
            in_maps.append({
                "qT": qT,
                "kT": kT,
                "kbT": kbT,
                "vT": vT,
                "wqT": np.ascontiguousarray(wq[:, hs]),
                "wkT": np.ascontiguousarray(wk[:, hs]),
                "wkbT": np.ascontiguousarray(wkb[:, hs]),
                "wvT": np.ascontiguousarray(wv[:, hs]),
                "woT": np.ascontiguousarray(wo[hs, :]),
                "bq": np.ascontiguousarray(inp["bq"][hs], dtype=f32),
                "bks": np.ascontiguousarray(
                    inp["bk"][hs] + inp["bkb"][hs], dtype=f32),
                "maskf": maskf,
            })
    return in_maps, TK


def gather(results, bo, bv_wo):
    out = np.empty((B, S, HID), np.float32)
    const = (np.asarray(bo, dtype=np.float32)
             + bv_wo[0] + bv_wo[1])
    for b in range(B):
        out[b] = (results[2 * b]["out"].astype(np.float32)
                  + results[2 * b + 1]["out"].astype(np.float32)
                  + const)
    return out


def bv_wo_terms(inputs):
    bv = np.asarray(inputs["bv"], dtype=np.float64)
    wo = np.asarray(inputs["Wo"], dtype=np.float64)
    return [
        (bv[g * DPC:(g + 1) * DPC] @ wo[:, g * DPC:(g + 1) * DPC].T)
        .astype(np.float32)
        for g in range(2)
    ]


_modules = {}
_executors = {}


def get_module(tk=S, reps=1):
    key = (tk, reps)
    if key not in _modules:
        _modules[key] = build_module(reps=reps, tk=tk)
    return _modules[key]


class _Executor:
    """Builds the SPMD PJRT executable once; later calls only move data."""

    def __init__(self, nc):
        import jax
        from jax.sharding import Mesh, PartitionSpec, NamedSharding
        from jax.experimental.shard_map import shard_map
        from concourse import bass2jax

        bass2jax.install_neuronx_cc_hook()
        self.jax = jax
        self.nc = nc
        pid = nc.partition_id_tensor.name if nc.partition_id_tensor else None
        in_names, out_names, out_avals, zeros = [], [], [], []
        for alloc in nc.m.functions[0].allocations:
            if not isinstance(alloc, mybir.MemoryLocationSet):
                continue
            name = alloc.memorylocations[0].name
            if alloc.kind == "ExternalInput":
                if name != pid:
                    in_names.append(name)
            elif alloc.kind == "ExternalOutput":
                out_names.append(name)
                shape = tuple(alloc.tensor_shape)
                dtype = mybir.dt.np(alloc.dtype)
                out_avals.append(jax.core.ShapedArray(shape, dtype))
                zeros.append(np.zeros(shape, dtype))
        self.in_names, self.out_names = in_names, out_names
        all_in = in_names + out_names + ([pid] if pid else [])

        def _body(*args):
            operands = list(args)
            if pid:
                operands.append(bass2jax.partition_id_tensor())
            return tuple(bass2jax._bass_exec_p.bind(
                *operands,
                out_avals=tuple(out_avals),
                in_names=tuple(all_in),
                out_names=tuple(out_names),
                lowering_input_output_aliases=(),
                sim_require_finite=True,
                sim_require_nnan=True,
                nc=nc,
            ))

        devices = jax.devices()[:NCORES]
        mesh = Mesh(np.asarray(devices), ("core",))
        spec = PartitionSpec("core")
        self.sharding = NamedSharding(mesh, spec)
        n_args = len(in_names) + len(out_names)
        self.fn = jax.jit(
            shard_map(_body, mesh=mesh, in_specs=(spec,) * n_args,
                      out_specs=(spec,) * len(out_names), check_rep=False),
            keep_unused=True,
        )
        self.zero_dev = [
            jax.device_put(
                np.zeros((NCORES * z.shape[0], *z.shape[1:]), z.dtype),
                self.sharding,
            )
            for z in zeros
        ]
        self.out_shapes = [tuple(a.shape) for a in out_avals]

    def run(self, in_maps):
        jax = self.jax
        dev_in = [
            jax.device_put(
                np.concatenate(
                    [np.asarray(in_maps[c][n]) for c in range(NCORES)], axis=0
                ),
                self.sharding,
            )
            for n in self.in_names
        ]
        outs = self.fn(*dev_in, *self.zero_dev)
        jax.block_until_ready(outs)
        results = []
        for c in range(NCORES):
            res = {}
            for i, n in enumerate(self.out_names):
                sh = self.out_shapes[i]
                res[n] = np.asarray(outs[i]).reshape(NCORES, *sh)[c]
            results.append(res)
        return results


def get_executor(tk=S, reps=1):
    key = (tk, reps)
    if key not in _executors:
        _executors[key] = _Executor(get_module(tk, reps))
    return _executors[key]


def kernel(**inputs):
    global _executors
    in_maps, tk = make_in_maps(inputs)
    last_err = None
    for attempt in range(3):
        try:
            if attempt < 2:
                res = get_executor(tk).run(in_maps)
            else:
                # fall back to the stock runner path
                res = run_bass_kernel_spmd(
                    get_module(tk), in_maps, core_ids=list(range(NCORES))
                ).results
            return gather(res, inputs["bo"], bv_wo_terms(inputs))
        except Exception as e:  # transient NRT/device errors: rebuild + retry
            last_err = e
            _executors = {}
            import time as _time
            _time.sleep(2.0 * (attempt + 1))
    raise last_err
